# revision 14
# baseline (speedup 1.0000x reference)
"""DeltaSynapse (gnn_message_passing) Trainium2 Bass kernel.

Computes I[b,o] = sum_e signs[e,o]*(W[e,o]*(1-f[e,o]) + Wlong[b,e,o]*f[e,o])
                  * Xpre[b,e,o],
with Xpre[b,e,o] = sum_d delaymap[d,e,o]*Xd[d,b,e]  (one-hot delay gather).

Strategy (8 NeuronCores): shard the postsynaptic axis o into 4 quarters of
512 and the presynaptic axis e into 2 halves of 1024; core (h,q) computes
the partial sum over its e-half for its o-quarter. The two e-half partials
are summed on the host (64KB) and the o-quarters concatenated.

Per core, all bulk inputs are host-packed into ONE contiguous array laid
out as [et, e128, slot, o] with 19 slots per e-row (8 delaymap d-slices,
8 Wlong b-slices, W, STDP_frac, signs), so each 128-e tile is a single
~4.9MB DMA with one contiguous descriptor per partition, cast f32->f16 by
the SDMA engines in flight.

On-device per e-tile (128 e x 512 o):
  - packed[e,d] = sum_b 2^b * Xd[d,b,e] (PE transpose + weighted reduce,
    once at setup).
  - Pi[e,o] = sum_d packed[e,d] * dmap[d,e,o] via PE (diag(packed[:,d])
    stationary built on the scalar engine); one-hot selection => Pi holds
    all 8 per-batch spike masks as an 8-bit integer (exact in f32 PSUM).
  - v[b] = signs*W*(1-f) + (signs*f)*Wlong[b]   (DVE, f16, 2x mode)
  - masks m01[b] = (int(Pi) >> b) & 1            (DVE, i16 4x)
  - m_f16 = cast(m01)                            (scalar engine)
  - t[b] = v[b] * m_f16[b]                       (DVE, f16 2x)
  - I[b,:] += column-sums of t[b] via PE matmul with one-hot-column
    stationary.
  The mask-dependent tail (cast, multiply, accumulate) for tile et is
  emitted in iteration et+1 (software pipelining) so no engine ever
  stalls on the PE->ACT mask roundtrip of the same tile.
"""
import numpy as np
from contextlib import ExitStack

D, B, N = 8, 8, 2048
NO = 512          # o columns per core
NE = 1024         # e rows per core
ET = NE // 128    # e-tiles per core
K = D + B + 3     # packed slots per e-row: dmap[0:8], wl[8:16], w, stdp, sgn
N_CORES = 8

_NC = None


def _build():
    from concourse import bacc, tile, mybir, masks
    from concourse.alu_op_type import AluOpType as op

    f32 = mybir.dt.float32
    f16 = mybir.dt.float16
    i16 = mybir.dt.int16
    Copy = mybir.ActivationFunctionType.Copy

    nc = bacc.Bacc("TRN2", target_bir_lowering=False, debug=False)

    big_d = nc.dram_tensor("big", (ET, 128, K, NO), f32, kind="ExternalInput")
    xd_d = nc.dram_tensor("xd", (D, B, NE), f32, kind="ExternalInput")
    out_d = nc.dram_tensor("iout", (B, NO), f32, kind="ExternalOutput")

    with tile.TileContext(nc) as tc, ExitStack() as ctx:
        cpool = ctx.enter_context(tc.tile_pool(name="const", bufs=1))
        lpool = ctx.enter_context(tc.tile_pool(name="loads", bufs=3))
        pool = ctx.enter_context(tc.tile_pool(name="work", bufs=2))
        pspool = ctx.enter_context(tc.tile_pool(name="pst", bufs=2, space="PSUM"))
        accpool = ctx.enter_context(tc.tile_pool(name="acc", bufs=1, space="PSUM"))

        # ---- first tiles' loads issue before the constant setup so the
        # DMA engines run during it
        pre = {}
        for et in range(3):
            t19 = lpool.tile([128, K, NO], f16, name=f"t19_{et}", tag="t19")
            nc.gpsimd.dma_start(t19[:], big_d[et])
            pre[et] = t19

        # ---- constants -------------------------------------------------
        ident = cpool.tile([D * B, D * B], f32)
        masks.make_identity(nc, ident[:])
        ebs = []
        for b in range(B):
            ebt = cpool.tile([128, B], f16, name=f"eb{b}")
            nc.vector.memset(ebt[:], 0.0)
            nc.vector.memset(ebt[:, b:b + 1], 1.0)
            ebs.append(ebt)
        pw = cpool.tile([128, D, B], f32)
        for b in range(B):
            nc.vector.memset(pw[:, :, b], float(1 << b))
        ident1 = cpool.tile([128, 128], f16)
        masks.make_identity(nc, ident1[:])

        # ---- pack Xd: packed[e, et, d] = sum_b 2^b * Xd[d, b, e] -------
        xd_nat = cpool.tile([D * B, NE], f32)
        nc.sync.dma_start(xd_nat[:], xd_d[:].flatten_outer_dims())
        packed = cpool.tile([128, ET, D], f32)
        for c in range(ET):
            xdt_ps = pspool.tile([128, D * B], f32, name=f"xdt{c}", tag="xdt")
            nc.tensor.matmul(
                xdt_ps[:], xd_nat[:, c * 128:(c + 1) * 128], ident[:],
                is_transpose=True)
            xw = pool.tile([128, D, B], f32, name=f"xw{c}", tag="xw")
            nc.vector.tensor_tensor(
                xw[:], xdt_ps[:].rearrange("e (d b) -> e d b", d=D), pw[:],
                op=op.mult)
            nc.vector.tensor_reduce(
                packed[:, c, :], xw[:], axis=mybir.AxisListType.X, op=op.add)

        acc = accpool.tile([B, NO], f32)

        # ---- main loop over e-tiles (mask tail pipelined one iter back) --
        carry = None  # (v_all, m_i16) of tile et-1
        for et in range(ET):
            if et in pre:
                t19 = pre[et]
            else:
                t19 = lpool.tile([128, K, NO], f16, tag="t19")
                nc.gpsimd.dma_start(t19[:], big_d[et])
            dm3 = t19[:, 0:D, :]
            wl3 = t19[:, D:D + B, :]
            w_t = t19[:, D + B, :]
            stdp_t = t19[:, D + B + 1, :]
            sgn_t = t19[:, D + B + 2, :]

            # prep: C = sgn*f; SW = sgn*w; An = (f-1)*SW = -A
            C_t = pool.tile([128, NO], f16, tag="C_t")
            nc.vector.tensor_tensor(C_t[:], sgn_t, stdp_t, op=op.mult)
            sw = pool.tile([128, NO], f16, tag="sw")
            nc.vector.tensor_tensor(sw[:], sgn_t, w_t, op=op.mult)
            An = pool.tile([128, NO], f16, tag="An")
            nc.vector.scalar_tensor_tensor(
                An[:], stdp_t, 1.0, sw[:], op0=op.subtract, op1=op.mult)

            # diag(packed[:,et,d]) stack on the scalar engine
            dstack = pool.tile([128, D, 128], f16, tag="dstack")
            for d in range(D):
                nc.scalar.activation(
                    dstack[:, d, :], ident1[:], Copy,
                    scale=packed[:, et, d].unsqueeze(-1))

            # Pi = sum_d diag(packed[:,et,d]) @ dmap[d] on the PE
            pi_ps = pspool.tile([128, NO], f32, name=f"pi_ps{et}", tag="pi_ps")
            for d in range(D):
                nc.tensor.matmul(
                    pi_ps[:], dstack[:, d, :], dm3[:, d, :],
                    start=(d == 0), stop=(d == D - 1))
            pi_i16 = pool.tile([128, NO], i16, tag="pi_i16")
            nc.scalar.activation(pi_i16[:], pi_ps[:], Copy)

            # v[b] = C*Wlong[b] - An  (= A + C*Wlong[b])  on the DVE
            v_all = pool.tile([128, B, NO], f16, tag="v_all")
            nc.vector.tensor_tensor(
                v_all[:], wl3,
                C_t[:].unsqueeze(1).broadcast_to((128, B, NO)), op=op.mult)
            nc.vector.tensor_tensor(
                v_all[:], v_all[:],
                An[:].unsqueeze(1).broadcast_to((128, B, NO)), op=op.subtract)

            # masks m01 = (pi >> b) & 1 in i16 (DVE, 4x mode)
            m_i16 = pool.tile([128, B, NO], i16, tag="m_i16")
            for b in range(B):
                nc.vector.tensor_scalar(
                    m_i16[:, b, :], pi_i16[:], b, 1,
                    op0=op.logical_shift_right, op1=op.bitwise_and)

            # ---- pipelined tail for tile et-1 ---------------------------
            if carry is not None:
                _tail(nc, pool, acc, ebs, carry, et - 1, False)
            carry = (v_all, m_i16)

        _tail(nc, pool, acc, ebs, carry, ET - 1, True)

        out_sb = cpool.tile([B, NO], f32)
        nc.scalar.activation(out_sb[:], acc[:], Copy)
        nc.sync.dma_start(out_d[:], out_sb[:])

    nc.compile()
    return nc


def _tail(nc, pool, acc, ebs, carry, et, last):
    """Mask cast + multiply + accumulate for tile et (pipelined)."""
    from concourse import mybir
    from concourse.alu_op_type import AluOpType as op
    f16 = mybir.dt.float16
    Copy = mybir.ActivationFunctionType.Copy
    v_all, m_i16 = carry
    m_f16 = pool.tile([128, B, NO], f16, tag="m_f16")
    nc.scalar.activation(m_f16[:], m_i16[:], Copy)
    t_all = pool.tile([128, B, NO], f16, tag="t_all")
    nc.vector.tensor_tensor(t_all[:], v_all[:], m_f16[:], op=op.mult)
    for b in range(B):
        nc.tensor.matmul(
            acc[:], ebs[b][:], t_all[:, b, :],
            start=(et == 0 and b == 0),
            stop=(last and b == B - 1))


def _in_maps(Xd, delaymap, W, Wlong, STDP_frac, signs):
    maps = []
    for c in range(N_CORES):
        h, q = divmod(c, 4)
        e0, o0 = h * NE, q * NO
        es, os_ = slice(e0, e0 + NE), slice(o0, o0 + NO)
        big = np.empty((ET, 128, K, NO), dtype=np.float32)
        # dmap slots 0:8  — big[et, e, d, o] = delaymap[d, e0+et*128+e, o0+o]
        big[:, :, 0:D, :] = np.asarray(delaymap[:, es, os_]).reshape(
            D, ET, 128, NO).transpose(1, 2, 0, 3)
        # wl slots 8:16
        big[:, :, D:D + B, :] = np.asarray(Wlong[:, es, os_]).reshape(
            B, ET, 128, NO).transpose(1, 2, 0, 3)
        big[:, :, D + B, :] = np.asarray(W[es, os_]).reshape(ET, 128, NO)
        big[:, :, D + B + 1, :] = np.asarray(
            STDP_frac[es, os_]).reshape(ET, 128, NO)
        big[:, :, D + B + 2, :] = np.asarray(
            signs[es, os_]).reshape(ET, 128, NO)
        maps.append({
            "big": big,
            "xd": np.ascontiguousarray(Xd[:, :, es]),
        })
    return maps


def _gather(outs):
    return np.concatenate(
        [outs[q] + outs[q + 4] for q in range(4)], axis=1).astype(np.float32)


def kernel(Xd, delaymap, W, Wlong, STDP_frac, signs):
    global _NC
    from concourse.bass_utils import run_bass_kernel_spmd
    if _NC is None:
        _NC = _build()
    maps = _in_maps(Xd, delaymap, W, Wlong, STDP_frac, signs)
    res = run_bass_kernel_spmd(_NC, maps, list(range(N_CORES)))
    return _gather([r["iout"] for r in res.results])
